# revision 28
# baseline (speedup 1.0000x reference)
"""Trainium2 Bass kernel for InterventionAwareStructure loss.

loss = sum_b,i,d A[b,i,d] * mask[regimes[b], d] / count   (scalar)

Data-parallel over batch across 8 NeuronCores. Each core:
  - streams its A shard [32, 512, 512] fp32 from HBM on the sync HWDGE
    ring at SDMA line rate (~27 GB/s/engine x 16): 7 chunks of 4 MB
    (32 KB per partition line) and the last chunk as 4 x 1 MB quarters
    so little work trails the final byte,
  - the idle Vector / GpSimd engines (alternating per chunk) do a
    first halving reduction in place (free-axis pair add), so TensorE
    only needs 8 fp32r one-hot matmuls per chunk (5.2 us) -- safely
    faster than the 9.3 us/chunk DMA stream, which therefore never
    stalls on compute,
  - chunks 0-6 accumulate into PSUM bank A whose 28 batch rows are
    drained and DMA'd out while the last chunk is still in flight;
    only the tail quarter's add + 2 matmuls + a [4, 512] store remain
    after the last byte,
  - the one-hot stationary table is computed on HOST and DMA'd in over
    the same ring (64 KB) before the stream warms up.

The mask gather (256x512), the mask dot, and the final scalar
reduction are all done on host; they are negligible next to the
256 MB stream of A.
"""

import numpy as np

import concourse.bass as bass
import concourse.tile as tile
from concourse import bacc, mybir
from concourse.bass_utils import run_bass_kernel_spmd

INTERVENTION_STRENGTH = 1.0

N_CORES = 8
B, N_REGIMES, D = 256, 16, 512
B_SH = B // N_CORES          # 32 batch items per core
NCHUNK = B_SH // 4           # 8 chunks of 4 batch items (4 MB fp32 each)
FREE = 4 * D * D // 128      # 8192 f32 per partition per chunk
HALF = FREE // 2             # free size after the halving add
QCOL = FREE // 4             # 2048-col (1 MB) quarters for the last chunk

_CACHED_NC = None
_W_HOST = None


def _build_w_host() -> np.ndarray:
    """One-hot block stationary table [128, NCHUNK*32] fp32.

    Chunk g holds batches 4g..4g+3; partition p carries rows of batch
    gb = p//32.  Block g routes partition p to PSUM row 4g + p//32.
    """
    w = np.zeros((128, NCHUNK * 32), dtype=np.float32)
    for g in range(NCHUNK - 1):
        for gb in range(4):
            w[gb * 32:(gb + 1) * 32, 32 * g + 4 * g + gb] = 1.0
    # The tail chunk maps to PSUM rows 0-3 so its [4, 512] result can
    # be copied from the 32-partition-aligned top of its own bank.
    for gb in range(4):
        w[gb * 32:(gb + 1) * 32, 32 * (NCHUNK - 1) + gb] = 1.0
    return w


def _build_nc() -> bass.Bass:
    nc = bacc.Bacc()
    f32 = mybir.dt.float32
    f32r = mybir.dt.float32r

    a = nc.dram_tensor("a", [B_SH, D, D], f32, kind="ExternalInput")
    # fp32 bits tagged fp32r so the weights' producer dtype satisfies
    # the BIR verifier without an on-device retag copy.
    w = nc.dram_tensor("w", [128, NCHUNK * 32], f32, kind="ExternalInput").bitcast(
        f32r
    )
    out = nc.dram_tensor("out", [B_SH, D], f32, kind="ExternalOutput")

    # chunk g of batches (4g..4g+3) -> SBUF [128, FREE]: partition
    # p = (gb * 32 + ih) holds rows i = ih*16 + il of batch 4g+gb; free
    # axis = (il, d) with a contiguous 32 KB line per partition.
    a_view = a.rearrange(
        "(ng gb) (ih il) d -> ng (gb ih) (il d)", ng=NCHUNK, ih=32
    )
    # Same bytes tagged fp32r: the last two 256 KB tail pieces skip the
    # DVE add and feed matmuls directly (a DMA producer passes the
    # fp32r verifier), so almost no work trails the final byte.
    ar_view = a.bitcast(f32r).rearrange(
        "(ng gb) (ih il) d -> ng (gb ih) (il d)", ng=NCHUNK, ih=32
    )

    mult = mybir.AluOpType.mult
    add = mybir.AluOpType.add

    with tile.TileContext(nc) as tc:
        with (
            tc.tile_pool(name="big", bufs=4) as big_pool,
            tc.tile_pool(name="scratch", bufs=3) as scratch_pool,
            tc.tile_pool(name="half", bufs=4) as half_pool,
            tc.tile_pool(name="ptail", bufs=1) as ptail_pool,
            tc.tile_pool(name="small", bufs=1) as small_pool,
            tc.tile_pool(name="psum", bufs=2, space="PSUM") as psum_pool,
        ):
            # W rides the scalar (ACT) HWDGE ring, whose preamble also
            # finishes earlier than sync's -- so chunk 0 starts there
            # too, buying ~2 us of stream head start.
            w_t = small_pool.tile([128, NCHUNK * 32], f32r)
            nc.scalar.dma_start(w_t[:], w[:, :])

            tiles = []
            for g in range(NCHUNK - 1):
                a_t = big_pool.tile([128, FREE], f32, tag="a")
                # 2 MB halves (16 KB lines): the DVE fold of a half can
                # start while the next half is still landing.  Chunk 0
                # rides SWDGE (gpsimd), whose engine preamble retires
                # ~2 us before sync's, so the stream starts earlier.
                ring = nc.gpsimd if g == 0 else nc.sync
                ring.dma_start(a_t[:, :HALF], a_view[g][:, :HALF])
                ring.dma_start(a_t[:, HALF:], a_view[g][:, HALF:])
                tiles.append(a_t)
            # Tail chunk: a 3 MB piece (24 KB lines) and a 0.5 MB piece
            # into the f32 tile (DVE-added like the others), then one
            # raw-f32r 0.5 MB piece that feeds 2 matmuls directly, so
            # almost no work trails the last byte.
            g7 = NCHUNK - 1
            a_t7 = big_pool.tile([128, FREE], f32, tag="a")
            for c0, c1 in ((0, 2048), (2048, 4096), (4096, 6144), (6144, 7168)):
                nc.sync.dma_start(a_t7[:, c0:c1], a_view[g7][:, c0:c1])
            p_t = ptail_pool.tile([128, 2 * D], f32r)
            nc.sync.dma_start(p_t[:, :D], ar_view[g7][:, 7168:7680])
            nc.sync.dma_start(p_t[:, D:], ar_view[g7][:, 7680:8192])

            def pair_add(out_ap, in0_ap, in1_ap):
                nc.vector.scalar_tensor_tensor(
                    out=out_ap, in0=in0_ap, scalar=1.0, in1=in1_ap,
                    op0=mult, op1=add,
                )

            # DVE folds each 2 MB half 4096 -> 512 with a chain of
            # contiguous pair-adds ping-ponging between the chunk tile
            # and a scratch tile (in-place adds miscompute on DVE; all
            # outs here are disjoint from their ins).  Full fp32 until
            # the last level, whose f32r destination is the rounding
            # "producer" the BIR verifier wants.  TensorE then needs
            # only one matmul per half; the ~4.6 us half chain fits
            # inside the ~4.85 us half landing period, so every stage
            # has slack and chunk 6 closes ~5 us before stream end.
            h4s = []
            for g in range(NCHUNK - 1):
                a_t = tiles[g]
                for hb in (0, HALF):
                    s_t = scratch_pool.tile([128, 2048], f32, tag="s")
                    pair_add(s_t[:], a_t[:, hb:hb + 2048],
                             a_t[:, hb + 2048:hb + 4096])
                    pair_add(a_t[:, hb:hb + 1024], s_t[:, :1024],
                             s_t[:, 1024:2048])
                    h_t = half_pool.tile([128, D], f32r, tag="h")
                    pair_add(h_t[:], a_t[:, hb:hb + D],
                             a_t[:, hb + D:hb + 2 * D])
                    h4s.append(h_t)

            # Chunks 0-6 accumulate into bank A (rows 0-27 of colsums);
            # it closes well before the stream ends so those rows
            # stream out while the tail chunk is still in flight.
            ps_a = psum_pool.tile([B_SH, D], f32, tag="psa")
            for k, h_t in enumerate(h4s):
                g = k // 2
                nc.tensor.matmul(
                    ps_a[:], w_t[:, g * 32:(g + 1) * 32], h_t[:],
                    start=(k == 0), stop=(k == len(h4s) - 1),
                )
            nbat = 4 * (NCHUNK - 1)
            o_a = small_pool.tile([nbat, D], f32)
            nc.scalar.copy(o_a[:], ps_a[:nbat, :])
            nc.scalar.dma_start(out[:nbat, :], o_a[:])

            # Tail chunk into bank B (rows 0-3 via its one-hot block):
            # each 1 MB quarter folds 2048 -> 512 then one matmul, all
            # pipelined against the quarter landings; the last 0.5 MB
            # rides raw f32r into 2 matmuls, so only ~2 matmuls + a
            # [4, 512] store trail the last byte.
            ps_b = psum_pool.tile([B_SH, D], f32, tag="psb")
            w_g = w_t[:, g7 * 32:(g7 + 1) * 32]
            mm_b = []
            for base, width in ((0, 2048), (2048, 2048), (4096, 2048),
                                (6144, 1024)):
                hq_t = half_pool.tile([128, D], f32r, tag="h")
                if width == 2048:
                    sq_t = scratch_pool.tile([128, 1024], f32, tag="s")
                    pair_add(sq_t[:], a_t7[:, base:base + 1024],
                             a_t7[:, base + 1024:base + 2048])
                    pair_add(hq_t[:], sq_t[:, :D], sq_t[:, D:2 * D])
                else:
                    pair_add(hq_t[:], a_t7[:, base:base + D],
                             a_t7[:, base + D:base + 2 * D])
                mm_b.append(hq_t[:])
            mm_b.append(p_t[:, :D])
            mm_b.append(p_t[:, D:])
            for k, mv in enumerate(mm_b):
                nc.tensor.matmul(
                    ps_b[:], w_g, mv,
                    start=(k == 0),
                    stop=(k == len(mm_b) - 1),
                )
            # Tail batches land in rows 0-3: a 32-partition-aligned
            # [4, 512] PSUM read, copied and stored as out rows 28-31.
            o_b = small_pool.tile([4, D], f32)
            nc.scalar.copy(o_b[:], ps_b[:4, :])
            nc.scalar.dma_start(out[nbat:, :], o_b[:])

    nc.finalize()
    return nc


def _get_nc() -> bass.Bass:
    global _CACHED_NC, _W_HOST
    if _CACHED_NC is None:
        _CACHED_NC = _build_nc()
        _W_HOST = _build_w_host()
    return _CACHED_NC


def _run(a_shards, **run_kwargs):
    nc = _get_nc()
    in_maps = [
        {"a": np.ascontiguousarray(a_shards[c]), "w": _W_HOST}
        for c in range(N_CORES)
    ]
    return run_bass_kernel_spmd(nc, in_maps, list(range(N_CORES)), **run_kwargs)


def kernel(A_per_env, intervention_mask, regimes, _run_kwargs=None):
    A_per_env = np.asarray(A_per_env, dtype=np.float32)
    intervention_mask = np.asarray(intervention_mask, dtype=np.float32)
    regs = np.asarray(regimes).astype(np.int64)

    n_regimes = intervention_mask.shape[0]
    valid = regs < n_regimes
    e = np.clip(regs, 0, n_regimes - 1)
    masks = intervention_mask[e] * valid[:, None].astype(np.float32)  # [B, D]

    a_shards = [A_per_env[c * B_SH:(c + 1) * B_SH] for c in range(N_CORES)]

    res = _run(a_shards, **(_run_kwargs or {}))
    num = np.float64(0.0)
    for c in range(N_CORES):
        colsums = res.results[c]["out"].astype(np.float64)        # [32, 512]
        num += (colsums * masks[c * B_SH:(c + 1) * B_SH]).sum()

    count = masks.astype(np.float64).sum()
    loss = num / count if count > 0 else num
    out = np.asarray(INTERVENTION_STRENGTH * loss, dtype=np.float32)
    if _run_kwargs is not None:
        return out, res
    return out


# revision 30
# speedup vs baseline: 1.0271x; 1.0271x over previous
"""Trainium2 Bass kernel for InterventionAwareStructure loss.

loss = sum_b,i,d A[b,i,d] * mask[regimes[b], d] / count   (scalar)

Data-parallel over batch across 8 NeuronCores. Each core:
  - streams its A shard [32, 512, 512] fp32 from HBM on the sync HWDGE
    ring at SDMA line rate (~27 GB/s/engine x 16): 7 chunks of 4 MB
    (32 KB per partition line) and the last chunk as 4 x 1 MB quarters
    so little work trails the final byte,
  - the idle Vector / GpSimd engines (alternating per chunk) do a
    first halving reduction in place (free-axis pair add), so TensorE
    only needs 8 fp32r one-hot matmuls per chunk (5.2 us) -- safely
    faster than the 9.3 us/chunk DMA stream, which therefore never
    stalls on compute,
  - chunks 0-6 accumulate into PSUM bank A whose 28 batch rows are
    drained and DMA'd out while the last chunk is still in flight;
    only the tail quarter's add + 2 matmuls + a [4, 512] store remain
    after the last byte,
  - the one-hot stationary table is computed on HOST and DMA'd in over
    the same ring (64 KB) before the stream warms up.

The mask gather (256x512), the mask dot, and the final scalar
reduction are all done on host; they are negligible next to the
256 MB stream of A.
"""

import numpy as np

import concourse.bass as bass
import concourse.tile as tile
from concourse import bacc, mybir
from concourse.bass_utils import run_bass_kernel_spmd

INTERVENTION_STRENGTH = 1.0

N_CORES = 8
B, N_REGIMES, D = 256, 16, 512
B_SH = B // N_CORES          # 32 batch items per core
NCHUNK = B_SH // 4           # 8 chunks of 4 batch items (4 MB fp32 each)
FREE = 4 * D * D // 128      # 8192 f32 per partition per chunk
HALF = FREE // 2             # free size after the halving add
QCOL = FREE // 4             # 2048-col (1 MB) quarters for the last chunk

_CACHED_NC = None
_W_HOST = None


def _build_w_host() -> np.ndarray:
    """One-hot block stationary table [128, NCHUNK*32] fp32.

    Chunk g holds batches 4g..4g+3; partition p carries rows of batch
    gb = p//32.  Block g routes partition p to PSUM row 4g + p//32.
    """
    w = np.zeros((128, NCHUNK * 32), dtype=np.float32)
    for g in range(NCHUNK - 1):
        for gb in range(4):
            w[gb * 32:(gb + 1) * 32, 32 * g + 4 * g + gb] = 1.0
    # The tail chunk maps to PSUM rows 0-3 so its [4, 512] result can
    # be copied from the 32-partition-aligned top of its own bank.
    for gb in range(4):
        w[gb * 32:(gb + 1) * 32, 32 * (NCHUNK - 1) + gb] = 1.0
    return w


def _build_nc() -> bass.Bass:
    nc = bacc.Bacc()
    f32 = mybir.dt.float32
    f32r = mybir.dt.float32r

    a = nc.dram_tensor("a", [B_SH, D, D], f32, kind="ExternalInput")
    # fp32 bits tagged fp32r so the weights' producer dtype satisfies
    # the BIR verifier without an on-device retag copy.
    w = nc.dram_tensor("w", [128, NCHUNK * 32], f32, kind="ExternalInput").bitcast(
        f32r
    )
    out = nc.dram_tensor("out", [B_SH, D], f32, kind="ExternalOutput")

    # chunk g of batches (4g..4g+3) -> SBUF [128, FREE]: partition
    # p = (gb * 32 + ih) holds rows i = ih*16 + il of batch 4g+gb; free
    # axis = (il, d) with a contiguous 32 KB line per partition.
    a_view = a.rearrange(
        "(ng gb) (ih il) d -> ng (gb ih) (il d)", ng=NCHUNK, ih=32
    )
    # Same bytes tagged fp32r: the last two 256 KB tail pieces skip the
    # DVE add and feed matmuls directly (a DMA producer passes the
    # fp32r verifier), so almost no work trails the final byte.
    ar_view = a.bitcast(f32r).rearrange(
        "(ng gb) (ih il) d -> ng (gb ih) (il d)", ng=NCHUNK, ih=32
    )

    mult = mybir.AluOpType.mult
    add = mybir.AluOpType.add

    with tile.TileContext(nc) as tc:
        with (
            tc.tile_pool(name="big", bufs=9) as big_pool,
            tc.tile_pool(name="scratch", bufs=2) as scratch_pool,
            tc.tile_pool(name="half", bufs=4) as half_pool,
            tc.tile_pool(name="ptail", bufs=1) as ptail_pool,
            tc.tile_pool(name="small", bufs=1) as small_pool,
            tc.tile_pool(name="psum", bufs=2, space="PSUM") as psum_pool,
        ):
            # W rides the scalar (ACT) HWDGE ring, whose preamble also
            # finishes earlier than sync's -- so chunk 0 starts there
            # too, buying ~2 us of stream head start.
            w_t = small_pool.tile([128, NCHUNK * 32], f32r)
            nc.scalar.dma_start(w_t[:], w[:, :])

            # One 2 MB half per tile, one dma_start per tile (16 KB
            # lines).  With 9 bufs the pool recycles ~4.5 chunks behind
            # the stream, so dma issue never couples to the DVE folds
            # (a bufs=4 whole-chunk layout showed a metastable slow
            # mode where issue -> landing -> fold -> recycle -> issue
            # locked ~30% above line rate).
            htiles = []
            for g in range(NCHUNK - 1):
                for hb in (0, 1):
                    a_t = big_pool.tile([128, HALF], f32, tag="a")
                    nc.sync.dma_start(
                        a_t[:], a_view[g][:, hb * HALF:(hb + 1) * HALF]
                    )
                    htiles.append(a_t)
            # Tail chunk: 1 MB quarters into two half tiles (folded
            # like the others, per quarter), then one raw-f32r 0.5 MB
            # piece feeding 2 matmuls directly, so almost no work
            # trails the last byte.
            g7 = NCHUNK - 1
            t_a = big_pool.tile([128, HALF], f32, tag="a")
            nc.sync.dma_start(t_a[:, :2048], a_view[g7][:, :2048])
            nc.sync.dma_start(t_a[:, 2048:], a_view[g7][:, 2048:4096])
            t_b = big_pool.tile([128, HALF], f32, tag="a")
            nc.sync.dma_start(t_b[:, :2048], a_view[g7][:, 4096:6144])
            nc.sync.dma_start(t_b[:, 2048:3072], a_view[g7][:, 6144:7168])
            p_t = ptail_pool.tile([128, 2 * D], f32r)
            nc.sync.dma_start(p_t[:, :D], ar_view[g7][:, 7168:7680])
            nc.sync.dma_start(p_t[:, D:], ar_view[g7][:, 7680:8192])

            def pair_add(out_ap, in0_ap, in1_ap):
                nc.vector.scalar_tensor_tensor(
                    out=out_ap, in0=in0_ap, scalar=1.0, in1=in1_ap,
                    op0=mult, op1=add,
                )

            # DVE folds each 2 MB half 4096 -> 512 with a chain of
            # contiguous pair-adds ping-ponging between the chunk tile
            # and a scratch tile (in-place adds miscompute on DVE; all
            # outs here are disjoint from their ins).  Full fp32 until
            # the last level, whose f32r destination is the rounding
            # "producer" the BIR verifier wants.  TensorE then needs
            # only one matmul per half; the ~4.6 us half chain fits
            # inside the ~4.85 us half landing period, so every stage
            # has slack and chunk 6 closes ~5 us before stream end.
            h4s = []
            for a_t in htiles:
                s_t = scratch_pool.tile([128, 2048], f32, tag="s")
                pair_add(s_t[:], a_t[:, :2048], a_t[:, 2048:4096])
                pair_add(a_t[:, :1024], s_t[:, :1024], s_t[:, 1024:2048])
                h_t = half_pool.tile([128, D], f32r, tag="h")
                pair_add(h_t[:], a_t[:, :D], a_t[:, D:2 * D])
                h4s.append(h_t)

            # Chunks 0-6 accumulate into bank A (rows 0-27 of colsums);
            # it closes well before the stream ends so those rows
            # stream out while the tail chunk is still in flight.
            ps_a = psum_pool.tile([B_SH, D], f32, tag="psa")
            for k, h_t in enumerate(h4s):
                g = k // 2
                nc.tensor.matmul(
                    ps_a[:], w_t[:, g * 32:(g + 1) * 32], h_t[:],
                    start=(k == 0), stop=(k == len(h4s) - 1),
                )
            nbat = 4 * (NCHUNK - 1)
            o_a = small_pool.tile([nbat, D], f32)
            nc.scalar.copy(o_a[:], ps_a[:nbat, :])
            nc.scalar.dma_start(out[:nbat, :], o_a[:])

            # Tail chunk into bank B (rows 0-3 via its one-hot block):
            # each 1 MB quarter folds 2048 -> 512 then one matmul, all
            # pipelined against the quarter landings; the last 0.5 MB
            # rides raw f32r into 2 matmuls, so only ~2 matmuls + a
            # [4, 512] store trail the last byte.
            ps_b = psum_pool.tile([B_SH, D], f32, tag="psb")
            w_g = w_t[:, g7 * 32:(g7 + 1) * 32]
            mm_b = []
            for src_t, base, width in ((t_a, 0, 2048), (t_a, 2048, 2048),
                                       (t_b, 0, 2048), (t_b, 2048, 1024)):
                hq_t = half_pool.tile([128, D], f32r, tag="h")
                if width == 2048:
                    sq_t = scratch_pool.tile([128, 1024], f32, tag="s")
                    pair_add(sq_t[:], src_t[:, base:base + 1024],
                             src_t[:, base + 1024:base + 2048])
                    pair_add(hq_t[:], sq_t[:, :D], sq_t[:, D:2 * D])
                else:
                    pair_add(hq_t[:], src_t[:, base:base + D],
                             src_t[:, base + D:base + 2 * D])
                mm_b.append(hq_t[:])
            mm_b.append(p_t[:, :D])
            mm_b.append(p_t[:, D:])
            for k, mv in enumerate(mm_b):
                nc.tensor.matmul(
                    ps_b[:], w_g, mv,
                    start=(k == 0),
                    stop=(k == len(mm_b) - 1),
                )
            # Tail batches land in rows 0-3: a 32-partition-aligned
            # [4, 512] PSUM read, copied and stored as out rows 28-31.
            o_b = small_pool.tile([4, D], f32)
            nc.scalar.copy(o_b[:], ps_b[:4, :])
            nc.scalar.dma_start(out[nbat:, :], o_b[:])

    nc.finalize()
    return nc


def _get_nc() -> bass.Bass:
    global _CACHED_NC, _W_HOST
    if _CACHED_NC is None:
        _CACHED_NC = _build_nc()
        _W_HOST = _build_w_host()
    return _CACHED_NC


def _run(a_shards, **run_kwargs):
    nc = _get_nc()
    in_maps = [
        {"a": np.ascontiguousarray(a_shards[c]), "w": _W_HOST}
        for c in range(N_CORES)
    ]
    return run_bass_kernel_spmd(nc, in_maps, list(range(N_CORES)), **run_kwargs)


def kernel(A_per_env, intervention_mask, regimes, _run_kwargs=None):
    A_per_env = np.asarray(A_per_env, dtype=np.float32)
    intervention_mask = np.asarray(intervention_mask, dtype=np.float32)
    regs = np.asarray(regimes).astype(np.int64)

    n_regimes = intervention_mask.shape[0]
    valid = regs < n_regimes
    e = np.clip(regs, 0, n_regimes - 1)
    masks = intervention_mask[e] * valid[:, None].astype(np.float32)  # [B, D]

    a_shards = [A_per_env[c * B_SH:(c + 1) * B_SH] for c in range(N_CORES)]

    res = _run(a_shards, **(_run_kwargs or {}))
    num = np.float64(0.0)
    for c in range(N_CORES):
        colsums = res.results[c]["out"].astype(np.float64)        # [32, 512]
        num += (colsums * masks[c * B_SH:(c + 1) * B_SH]).sum()

    count = masks.astype(np.float64).sum()
    loss = num / count if count > 0 else num
    out = np.asarray(INTERVENTION_STRENGTH * loss, dtype=np.float32)
    if _run_kwargs is not None:
        return out, res
    return out
